# revision 35
# baseline (speedup 1.0000x reference)
"""Trainium2 Bass kernel for nn_EnhancedTAGAttention.

Reference computation:
    qn/kn/vn = LayerNorm(query/key/value) (shared gamma/beta)
    q = qn @ Wq.T + bq (16 heads x 64), k, v analogous
    scores = (q @ k.T)/8 + weighted * sigmoid(rel_scale) * 0.1
        weighted[b,q,k] = sum_c tag_relations[b,q,k,c] * softmax(rel_type_w)[c]
    attn_weights = softmax(scores)   [mask is all-ones per problem spec -> no-op]
    attn_output  = (attn_weights @ v) @ Wo.T + bo
    returns (attn_output, attn_weights)

Sharding: 8 cores = (batch b in {0,1}) x (query-block j in {0..3}, 512 q-rows).
Each core handles all 16 heads for its (b, q-block): full k/v for that batch.
Per-core outputs: attn_output rows [512, 1024] f32 and attn_weights in
head-transposed bf16 layout awT[h, k, q]; the host assembles/transposes/upcasts.

Device design (one SPMD program, all-bf16 PE path):
  - Host folds: gamma/beta into W/b; 1/sqrt(dh) into Wq/bq; bv into bo
    (attn rows sum to 1); W passed pre-transposed [d, o] in bf16;
    coeff = softmax(rel_type_w)*sigmoid(rel_scale)*0.1.
  - LayerNorm natural (tokens on partitions, bn_stats/bn_aggr;
    rstd = exp(-0.5*ln(var+eps)) so every ACT call stays in the
    natural_log_exp table set -> single ACT table load for the whole kernel).
  - z tiles PE-transposed (bf16) so projections contract d on partitions:
    qT/kT produced [o, s]; v produced natural [s, o] into v_aug blocks of 65
    columns (64 v-cols + a ones column).
  - Relation bias: weighted^T computed once (DVE mul+reduce, PE transpose)
    and stored as exp(weighted^T) bf16, interleaved into the v-projection loop.
  - Per head: scoresT[k, q] on PE -> ACT exp straight from PSUM (bf16 out) ->
    DVE multiply by exp(weighted^T) per quad (exp(a+b) = exp(a)*exp(b)).
    The @v matmul contracts k with the ones column appended, so the softmax
    denominator Z lands in PSUM row 64 for free.  1/Z = exp(-ln Z) on ACT,
    replicated across partitions by a 1-deep (-1)-matmul.  attn rows and E
    are normalized on DVE; E goes straight to HBM as bf16.
  - Output projection consumes attn_concatT [d, q] with prefetched Wo^T.

Measured on trn2 (8 cores, axon): ~520-540 us per core, rel err ~5e-3
(bf16 quantization; fp32 LN stats and softmax accumulation throughout).
"""

import math
import os

import ml_dtypes
import numpy as np

# ---- problem constants (hardcoded by contract) -----------------------------
B, S, D = 2, 2048, 1024
H, DH = 16, 64
P = 128
NQB = 4  # q-blocks per batch -> 8 cores = B * NQB
QB = S // NQB  # 512
EPS = 1e-5
NCORES = 8

_PROGRAM_CACHE = {}
LAST_RESULTS = None  # BassKernelResults of most recent run (for test harness)


def _build_program():
    import concourse.bass as bass
    import concourse.mybir as mybir
    import concourse.tile as tile
    from concourse import bacc
    from concourse.masks import make_identity

    dt = mybir.dt
    Alu = mybir.AluOpType
    Act = mybir.ActivationFunctionType
    f32, bf16 = dt.float32, dt.bfloat16

    nc = bacc.Bacc(target_bir_lowering=False)

    # ---- I/O ---------------------------------------------------------------
    xq = nc.dram_tensor("xq", [QB, D], f32, kind="ExternalInput")
    xk = nc.dram_tensor("xk", [S, D], f32, kind="ExternalInput")
    xv = nc.dram_tensor("xv", [S, D], f32, kind="ExternalInput")
    tag = nc.dram_tensor("tag", [QB, S * 4], f32, kind="ExternalInput")
    wqT = nc.dram_tensor("wqT", [D, D], bf16, kind="ExternalInput")
    wkT = nc.dram_tensor("wkT", [D, D], bf16, kind="ExternalInput")
    wvT = nc.dram_tensor("wvT", [D, D], bf16, kind="ExternalInput")
    woT = nc.dram_tensor("woT", [D, D], bf16, kind="ExternalInput")
    bq = nc.dram_tensor("bq", [D], f32, kind="ExternalInput")
    bk = nc.dram_tensor("bk", [D], f32, kind="ExternalInput")
    bo = nc.dram_tensor("bo", [D], f32, kind="ExternalInput")
    coeff = nc.dram_tensor("coeff", [1, 4], f32, kind="ExternalInput")

    awT = nc.dram_tensor("awT", [H, S, QB], bf16, kind="ExternalOutput")
    attn_out = nc.dram_tensor("attn_out", [QB, D], f32, kind="ExternalOutput")

    VA = DH + 1  # v_aug block: 64 v-cols + ones column (fused softmax denom)

    with tile.TileContext(nc) as tc:
        with (
            tc.tile_pool(name="singles", bufs=1) as singles,
            tc.tile_pool(name="persist", bufs=1) as persist,
        ):
            # ---- constants -------------------------------------------------
            identity = singles.tile([P, P], bf16)
            make_identity(nc, identity)
            eps_t = singles.tile([P, 1], f32)
            nc.vector.memset(eps_t, EPS)
            minus1f = singles.tile([1, P], f32)
            nc.vector.memset(minus1f, -1.0)
            coeff_t = singles.tile([P, 4], f32)
            bq_t = singles.tile([P, 8], f32)
            bk_t = singles.tile([P, 8], f32)
            bo_t = singles.tile([P, D], f32)

            def load_constants():
                # issued after the first x-loads so the LayerNorm critical
                # path is not queued behind constant DMAs at startup
                nc.gpsimd.dma_start(
                    out=coeff_t,
                    in_=bass.AP(tensor=coeff, offset=0, ap=[[0, P], [1, 4]]),
                )
                nc.sync.dma_start(bq_t, bq[:].rearrange("(c p) -> p c", p=P))
                nc.sync.dma_start(bk_t, bk[:].rearrange("(c p) -> p c", p=P))
                nc.gpsimd.dma_start(
                    out=bo_t,
                    in_=bass.AP(tensor=bo, offset=0, ap=[[0, P], [1, D]]),
                )

            # ---- persistent SBUF tensors -----------------------------------
            expWT = persist.tile([P, 16 * QB], bf16)   # exp(weighted^T) (kchunk, q)
            qT_proj = persist.tile([P, 8 * QB], bf16)  # (ochunk, q)
            kT_proj = persist.tile([P, 8 * S], bf16)   # (ochunk, k)
            v_aug = persist.tile([P, 16 * H * VA], bf16)  # (ktile, head, 64+1)
            # only the per-head "ones" columns need initialising
            nc.vector.memset(
                v_aug[:].rearrange("p (t h a) -> p t h a", t=16, a=VA)[:, :, :, DH : DH + 1],
                1.0,
            )
            acT = persist.tile([P, 8 * QB], bf16)      # (dchunk, q)

            # ================= LN + q/k/v projections =======================
            def layernorm_tile(pool_x, pool_sm, x_dram, row0):
                x_t = pool_x.tile([P, D], f32, tag="x")
                nc.sync.dma_start(x_t, x_dram[row0 : row0 + P, :])
                stats = pool_sm.tile([P, 2, 6], f32, tag="stats")
                nc.vector.bn_stats(stats[:, 0, :], x_t[:, 0:512])
                nc.vector.bn_stats(stats[:, 1, :], x_t[:, 512:1024])
                mv = pool_sm.tile([P, 2], f32, tag="mv")
                nc.vector.bn_aggr(mv[:], stats[:])
                # rstd = exp(-0.5*ln(var+eps)): keeps every ACT call in the
                # natural_log_exp set -> no ACT table reloads anywhere
                lnv = pool_sm.tile([P, 1], f32, tag="lnv")
                nc.scalar.activation(lnv, mv[:, 1:2], Act.Ln, bias=eps_t[:, 0:1])
                rstd = pool_sm.tile([P, 1], f32, tag="rstd")
                nc.scalar.activation(rstd, lnv, Act.Exp, scale=-0.5)
                z_t = pool_x.tile([P, D], bf16, tag="z")
                nc.vector.tensor_scalar(
                    z_t[:], x_t[:], mv[:, 0:1], rstd[:, 0:1], Alu.subtract, Alu.mult
                )
                return z_t

            def transpose_into(zt, dest3, s_idx, psum_pool):
                for g in range(2):  # groups of 4 d-chunks per psum bank
                    pt = psum_pool.tile([P, 4, P], bf16, tag="pt")
                    for j in range(4):
                        nc.tensor.transpose(
                            pt[:, j, :],
                            zt[:, (4 * g + j) * P : (4 * g + j + 1) * P],
                            identity,
                        )
                    nc.scalar.copy(
                        dest3[:, 4 * g : 4 * g + 4, s_idx * P : (s_idx + 1) * P],
                        pt[:],
                    )

            with (
                tc.tile_pool(name="xp", bufs=2) as xp,
                tc.tile_pool(name="smp", bufs=4) as smp,
                tc.tile_pool(name="tagp", bufs=1) as tagp,
                tc.tile_pool(name="wnat", bufs=2) as wnatp,
                tc.tile_pool(name="wp", bufs=16) as wp,
                tc.tile_pool(name="zp", bufs=1) as zp,
                tc.tile_pool(name="zsl", bufs=2) as zsl,
                tc.tile_pool(name="psT", bufs=3, space="PSUM") as psT,
                tc.tile_pool(name="psP", bufs=3, space="PSUM") as psP,
            ):
                # ---- q ----
                zqT = zp.tile([P, 8 * QB], bf16)
                zqT3 = zqT[:].rearrange("p (c s) -> p c s", s=QB)
                for t in range(QB // P):  # 4
                    z_t = layernorm_tile(xp, smp, xq, t * P)
                    transpose_into(z_t, zqT3, t, psT)
                load_constants()
                # prefetch q/k projection weights (after the q x-loads, so the
                # first LayerNorms aren't queued behind 16 weight DMAs)
                wq_sb = [wp.tile([P, D], bf16, tag="w", name=f"wq{_i}") for _i in range(8)]
                wk_sb = [wp.tile([P, D], bf16, tag="w", name=f"wk{_i}") for _i in range(8)]
                for dc in range(8):
                    nc.sync.dma_start(wq_sb[dc], wqT[dc * P : (dc + 1) * P, :])
                for dc in range(8):
                    nc.sync.dma_start(wk_sb[dc], wkT[dc * P : (dc + 1) * P, :])
                for oc in range(8):
                    pp = psP.tile([P, QB], f32, tag="pp")
                    for dc in range(8):
                        nc.tensor.matmul(
                            pp,
                            wq_sb[dc][:, oc * P : (oc + 1) * P],
                            zqT[:, dc * QB : (dc + 1) * QB],
                            start=(dc == 0),
                            stop=(dc == 7),
                        )
                    nc.scalar.activation(
                        qT_proj[:, oc * QB : (oc + 1) * QB],
                        pp,
                        Act.Identity,
                        bias=bq_t[:, oc : oc + 1],
                    )

                # ---- tag relation bias -> expWT, interleaved with k below ----
                wT3 = expWT[:].rearrange("p (c q) -> p c q", q=QB)

                def tag_iter(qt):
                    tag_t = tagp.tile([P, S * 4], bf16, tag="tag")
                    nc.gpsimd.dma_start(out=tag_t[:], in_=tag[qt * P : (qt + 1) * P, :])
                    tag3 = tag_t[:].rearrange("p (s c) -> p s c", c=4)
                    w_nat = wnatp.tile([P, S], bf16, tag="wnat")
                    for half in range(2):
                        t3 = tag3[:, half * (S // 2) : (half + 1) * (S // 2), :]
                        cpat = coeff_t[:, None, :].to_broadcast((P, S // 2, 4))
                        nc.vector.tensor_tensor(t3, t3, cpat, Alu.mult)
                        with nc.allow_low_precision(reason="4-way add of small bf16 bias"):
                            nc.vector.tensor_reduce(
                                w_nat[:, half * (S // 2) : (half + 1) * (S // 2)],
                                t3,
                                axis=mybir.AxisListType.X,
                                op=Alu.add,
                            )
                    for g in range(4):  # groups of 4 kchunks per psum bank
                        pt = psT.tile([P, 4, P], bf16, tag="pt")
                        for j in range(4):
                            nc.tensor.transpose(
                                pt[:, j, :],
                                w_nat[:, (4 * g + j) * P : (4 * g + j + 1) * P],
                                identity,
                            )
                        nc.scalar.activation(
                            wT3[:, 4 * g : 4 * g + 4, qt * P : (qt + 1) * P],
                            pt[:],
                            Act.Exp,
                        )

                # ---- k ----
                wv_sb = [wp.tile([P, D], bf16, tag="w", name=f"wv{_i}") for _i in range(8)]
                for dc in range(8):
                    nc.sync.dma_start(wv_sb[dc], wvT[dc * P : (dc + 1) * P, :])
                for sl in range(S // 512):  # 4
                    zkT = zsl.tile([P, 8 * 512], bf16, tag="zk")
                    zkT3 = zkT[:].rearrange("p (c s) -> p c s", s=512)
                    for t in range(4):
                        z_t = layernorm_tile(xp, smp, xk, sl * 512 + t * P)
                        transpose_into(z_t, zkT3, t, psT)
                    for oc in range(8):
                        pp = psP.tile([P, 512], f32, tag="pp")
                        for dc in range(8):
                            nc.tensor.matmul(
                                pp,
                                wk_sb[dc][:, oc * P : (oc + 1) * P],
                                zkT[:, dc * 512 : (dc + 1) * 512],
                                start=(dc == 0),
                                stop=(dc == 7),
                            )
                        nc.scalar.activation(
                            kT_proj[:, oc * S + sl * 512 : oc * S + (sl + 1) * 512],
                            pp,
                            Act.Identity,
                            bias=bk_t[:, oc : oc + 1],
                        )

                # ---- v ---- (natural layout into v_aug; bv folded into bo on host)
                va3 = v_aug[:].rearrange("p (t h a) -> p (t h) a", t=16, a=VA)
                for kt in range(S // P):  # 16
                    if kt % 4 == 0:
                        tag_iter(kt // 4)
                    z_t = layernorm_tile(xp, smp, xv, kt * P)
                    zvT = zsl.tile([P, 8 * P], bf16, tag="zv")
                    zvT3 = zvT[:].rearrange("p (c s) -> p c s", s=P)
                    transpose_into(z_t, zvT3, 0, psT)
                    for oh in range(2):
                        pp = psP.tile([P, 512], f32, tag="pp")
                        for dc in range(8):
                            nc.tensor.matmul(
                                pp,
                                zvT[:, dc * P : (dc + 1) * P],
                                wv_sb[dc][:, oh * 512 : (oh + 1) * 512],
                                start=(dc == 0),
                                stop=(dc == 7),
                            )
                        nc.scalar.copy(
                            va3[:, kt * H + oh * 8 : kt * H + (oh + 1) * 8, 0:DH],
                            pp[:].rearrange("p (h a) -> p h a", a=DH),
                        )

            # ============ attention (incl. tag bias + output proj weights) ==
            NK = S // P  # 16 k-chunks
            wo_pool_cm = tc.tile_pool(name="wo_p", bufs=8)
            wo_p = wo_pool_cm.__enter__()
            with (
                tc.tile_pool(name="Ep", bufs=3) as Ep,
                tc.tile_pool(name="rzp", bufs=3) as rzp,
                tc.tile_pool(name="psS", bufs=2, space="PSUM") as psS,
                tc.tile_pool(name="psAV", bufs=3, space="PSUM") as psAV,
                tc.tile_pool(name="psA", bufs=1, space="PSUM") as psA,
            ):
                # prefetch output-projection weights during attention
                wo_sb = [wo_p.tile([P, D], bf16, tag="wo", name=f"wo{_i}") for _i in range(8)]
                for dc in range(8):
                    nc.sync.dma_start(wo_sb[dc], woT[dc * P : (dc + 1) * P, :])

                def head_front(h):
                    """scores -> exp -> *expWT -> @v, quad by quad.

                    Interleaving the @v accumulation into the scores loop means
                    the softmax denominator is ready right after the last exp,
                    so the ACT does not idle waiting for a monolithic @v block.
                    Returns (E tile, pav psum)."""
                    hp = 64 * (h % 2)
                    oc = h // 2
                    qT_h = qT_proj[hp : hp + 64, oc * QB : (oc + 1) * QB]
                    E = Ep.tile([P, NK * QB], bf16, tag="E", name=f"E{h}")
                    pav = psAV.tile([VA, QB], f32, tag="pav")
                    for g in range(8):  # pairs of k-chunks
                        ps = psS.tile([P, 2 * QB], f32, tag="ps")
                        for i in range(2):
                            c = 2 * g + i
                            nc.tensor.matmul(
                                ps[:, i * QB : (i + 1) * QB],
                                kT_proj[hp : hp + 64, oc * S + c * P : oc * S + (c + 1) * P],
                                qT_h,
                                start=True,
                                stop=True,
                            )
                        nc.scalar.activation(
                            E[:, 2 * g * QB : 2 * (g + 1) * QB], ps[:], Act.Exp
                        )
                        if g % 2 == 1:  # quad complete -> fold in exp(weighted)
                            q0 = (2 * g - 2) * QB
                            q1 = (2 * g + 2) * QB
                            nc.vector.tensor_tensor(
                                E[:, q0:q1], E[:, q0:q1], expWT[:, q0:q1], Alu.mult
                            )
                    return E, pav

                def head_tail(h, E, pav):
                    """@v + softmax denom + normalizations + output DMA."""
                    hp = 64 * (h % 2)
                    oc = h // 2
                    E3 = E[:].rearrange("p (c q) -> p c q", q=QB)
                    for c in range(NK):
                        nc.tensor.matmul(
                            pav,
                            v_aug[:, (c * H + h) * VA : (c * H + h + 1) * VA],
                            E[:, c * QB : (c + 1) * QB],
                            start=(c == 0),
                            stop=(c == NK - 1),
                        )
                    # 1/Z replicated across partitions: exp(-ln Z), broadcast
                    # via a tiny f32 ones-matmul
                    lnz1 = rzp.tile([1, QB], f32, tag="lnz1")
                    nc.scalar.activation(lnz1, pav[DH : DH + 1, :], Act.Ln)
                    rzPS = psA.tile([P, QB], f32, tag="rzPS")
                    nc.tensor.matmul(rzPS, minus1f, lnz1, start=True, stop=True)
                    rzb = rzp.tile([P, QB], bf16, tag="rzb")
                    nc.scalar.activation(rzb, rzPS, Act.Exp)
                    nc.vector.tensor_tensor(
                        acT[hp : hp + 64, oc * QB : (oc + 1) * QB],
                        pav[0:DH, :],
                        rzb[0:DH, :],
                        Alu.mult,
                    )
                    awT3 = awT[h].rearrange("(c p) q -> p c q", p=P)
                    half = NK // 2
                    for j in range(2):
                        sl = E3[:, j * half : (j + 1) * half, :]
                        nc.vector.tensor_tensor(
                            sl, sl, rzb[:, None, :].to_broadcast((P, half, QB)), Alu.mult
                        )
                        nc.sync.dma_start(awT3[:, j * half : (j + 1) * half, :], sl)

                for h in range(H):
                    E, pav = head_front(h)
                    head_tail(h, E, pav)

            # ================= output projection ============================
            with (
                tc.tile_pool(name="outp", bufs=3) as outp,
                tc.tile_pool(name="psO", bufs=2, space="PSUM") as psO,
            ):
                for qc in range(QB // P):  # 4
                    for oh in range(2):
                        po = psO.tile([P, 512], f32, tag="po")
                        for dc in range(8):
                            nc.tensor.matmul(
                                po,
                                acT[:, dc * QB + qc * P : dc * QB + (qc + 1) * P],
                                wo_sb[dc][:, oh * 512 : (oh + 1) * 512],
                                start=(dc == 0),
                                stop=(dc == 7),
                            )
                        o_sb = outp.tile([P, 512], f32, tag="osb")
                        nc.vector.tensor_tensor(
                            o_sb,
                            po,
                            bo_t[:, oh * 512 : (oh + 1) * 512],
                            Alu.add,
                        )
                        nc.sync.dma_start(
                            attn_out[qc * P : (qc + 1) * P, oh * 512 : (oh + 1) * 512],
                            o_sb,
                        )
            wo_pool_cm.__exit__(None, None, None)

    # Force Exp/Ln to resolve to natural_log_exp_and_others (the only set
    # holding both) so the ACT never reloads tables mid-kernel.  Set indices
    # must stay aligned with act_info.json, so edit contents, not order.
    import concourse.bacc as _bacc_mod
    from concourse.hw_specs import get_activation_tables as _gat

    def _gat_patched(arch):
        t = dict(_gat(arch))
        for name in ("exp_and_others", "exp_and_friends"):
            if name in t:
                t[name] = t[name] - {Act.Exp}
        if "natural_log" in t:
            t["natural_log"] = t["natural_log"] - {Act.Ln}
        return t

    orig = _bacc_mod.get_activation_tables
    _bacc_mod.get_activation_tables = _gat_patched
    try:
        nc.finalize()
    finally:
        _bacc_mod.get_activation_tables = orig
    return nc


def _get_program():
    if "nc" not in _PROGRAM_CACHE:
        _PROGRAM_CACHE["nc"] = _build_program()
    return _PROGRAM_CACHE["nc"]


def _install_axon_trace_support():
    """Register the NTFF-profile hook that concourse's axon trace path expects.

    The agent image lacks ``antenv.axon_hooks``; replicate trn_boot's ctypes
    hook against the local libaxon_pjrt.so.  Profiling-only; inert unless
    KERNEL_TRACE=1.
    """
    import contextlib
    import ctypes
    import sys
    import types

    if "antenv.axon_hooks" in sys.modules:
        return
    try:
        import antenv
    except ImportError:
        return
    so_path = "/opt/axon/libaxon_pjrt.so"
    if not os.path.exists(so_path):
        return
    lib = ctypes.CDLL(so_path)
    if not hasattr(lib, "axon_start_nrt_profile"):
        return
    lib.axon_start_nrt_profile.argtypes = [
        ctypes.POINTER(ctypes.c_int64),
        ctypes.c_size_t,
    ]
    lib.axon_start_nrt_profile.restype = ctypes.c_int64
    lib.axon_stop_nrt_profile.argtypes = [ctypes.c_char_p]
    lib.axon_stop_nrt_profile.restype = ctypes.c_int64

    @contextlib.contextmanager
    def _hook(output_dir, device_ids):
        import jax

        jax.devices()
        if device_ids:
            ids = (ctypes.c_int64 * len(device_ids))(*device_ids)
            rc = lib.axon_start_nrt_profile(ids, len(device_ids))
        else:
            rc = lib.axon_start_nrt_profile(None, 0)
        if rc != 0:
            raise RuntimeError(f"axon_start_nrt_profile rc={rc}")
        try:
            yield
        finally:
            n = lib.axon_stop_nrt_profile(str(output_dir).encode())
            print(f"ntff profile: {n} file(s) written to {output_dir}")

    hooks = types.ModuleType("antenv.axon_hooks")
    _store = {"h": _hook}
    hooks.set_axon_ntff_profile_hook = lambda h: _store.__setitem__("h", h)
    hooks.get_axon_ntff_profile_hook = lambda: _store["h"]
    sys.modules["antenv.axon_hooks"] = hooks
    antenv.axon_hooks = hooks

    # avoid S3 artifact uploads from the profile path in this container
    import concourse.bass_utils as bu

    bu.upload_artifacts = lambda tmpdir: tmpdir


def kernel(**inputs):
    global LAST_RESULTS
    from concourse.bass_utils import run_bass_kernel_spmd

    q = np.asarray(inputs["query"], np.float32)
    k = np.asarray(inputs["key"], np.float32)
    v = np.asarray(inputs["value"], np.float32)
    tag = np.asarray(inputs["tag_relations"], np.float32)
    gamma = np.asarray(inputs["ln_gamma"], np.float32)
    beta = np.asarray(inputs["ln_beta"], np.float32)
    Wq = np.asarray(inputs["Wq"], np.float32)
    Wk = np.asarray(inputs["Wk"], np.float32)
    Wv = np.asarray(inputs["Wv"], np.float32)
    Wo = np.asarray(inputs["Wo"], np.float32)
    bq = np.asarray(inputs["bq"], np.float32)
    bk = np.asarray(inputs["bk"], np.float32)
    bv = np.asarray(inputs["bv"], np.float32)
    bo = np.asarray(inputs["bo"], np.float32)
    rel_type_w = np.asarray(inputs["rel_type_w"], np.float32)
    rel_scale = np.asarray(inputs["rel_scale"], np.float32)
    # NOTE: mask is all-ones per the problem spec (fill: ones) -> no-op.

    scale = 1.0 / math.sqrt(DH)
    # fold LN gamma/beta and the 1/sqrt(dh) scale into weights/biases (host-side
    # static param prep; gamma/beta are per-feature so they fold exactly).
    f64 = np.float64
    bf16 = ml_dtypes.bfloat16
    wqT = np.ascontiguousarray((gamma[:, None] * Wq.T.astype(f64)) * scale).astype(bf16)
    bq_e = np.asarray((bq + Wq.astype(f64) @ beta) * scale, np.float32)
    wkT = np.ascontiguousarray(gamma[:, None] * Wk.T.astype(f64)).astype(bf16)
    bk_e = np.asarray(bk + Wk.astype(f64) @ beta, np.float32)
    wvT = np.ascontiguousarray(gamma[:, None] * Wv.T.astype(f64)).astype(bf16)
    bv_e = np.asarray(bv + Wv.astype(f64) @ beta, f64)
    woT = np.ascontiguousarray(Wo.T).astype(bf16)
    # attn weights sum to 1, so the (v + bv) bias passes through attention
    # unchanged and folds into the output bias: bo_eff = bo + Wo @ bv
    bo = np.asarray(bo + Wo.astype(f64) @ bv_e, np.float32)

    rw = np.exp(rel_type_w - rel_type_w.max())
    rw /= rw.sum()
    coeff = (rw * (1.0 / (1.0 + np.exp(-rel_scale))) * 0.1).astype(np.float32)
    coeff = np.ascontiguousarray(coeff.reshape(1, 4))

    shared = dict(
        wqT=wqT, wkT=wkT, wvT=wvT, woT=woT,
        bq=bq_e, bk=bk_e, bo=bo, coeff=coeff,
    )
    in_maps = []
    for core in range(NCORES):
        b, j = divmod(core, NQB)
        in_maps.append(
            dict(
                shared,
                xq=np.ascontiguousarray(q[b, j * QB : (j + 1) * QB]),
                xk=np.ascontiguousarray(k[b]),
                xv=np.ascontiguousarray(v[b]),
                tag=np.ascontiguousarray(
                    tag[b, j * QB : (j + 1) * QB].reshape(QB, S * 4)
                ),
            )
        )

    nc = _get_program()
    trace = bool(int(os.environ.get("KERNEL_TRACE", "0")))
    if trace:
        _install_axon_trace_support()
    res = run_bass_kernel_spmd(
        nc, in_maps, core_ids=list(range(NCORES)), trace=trace
    )
    LAST_RESULTS = res

    attn_output = np.empty((B, S, D), np.float32)
    attn_weights = np.empty((B, H, S, S), np.float32)
    for core in range(NCORES):
        b, j = divmod(core, NQB)
        r = res.results[core]
        attn_output[b, j * QB : (j + 1) * QB, :] = r["attn_out"]
        # awT[h, k, q] (bf16) -> [h, q, k] fp32
        attn_weights[b, :, j * QB : (j + 1) * QB, :] = (
            r["awT"].astype(np.float32).swapaxes(1, 2)
        )
    return attn_output, attn_weights


# revision 36
# speedup vs baseline: 1.0093x; 1.0093x over previous
"""Trainium2 Bass kernel for nn_EnhancedTAGAttention.

Reference computation:
    qn/kn/vn = LayerNorm(query/key/value) (shared gamma/beta)
    q = qn @ Wq.T + bq (16 heads x 64), k, v analogous
    scores = (q @ k.T)/8 + weighted * sigmoid(rel_scale) * 0.1
        weighted[b,q,k] = sum_c tag_relations[b,q,k,c] * softmax(rel_type_w)[c]
    attn_weights = softmax(scores)   [mask is all-ones per problem spec -> no-op]
    attn_output  = (attn_weights @ v) @ Wo.T + bo
    returns (attn_output, attn_weights)

Sharding: 8 cores = (batch b in {0,1}) x (query-block j in {0..3}, 512 q-rows).
Each core handles all 16 heads for its (b, q-block): full k/v for that batch.
Per-core outputs: attn_output rows [512, 1024] f32 and attn_weights in
head-transposed bf16 layout awT[h, k, q]; the host assembles/transposes/upcasts.

Device design (one SPMD program, all-bf16 PE path):
  - Host folds: gamma/beta into W/b; 1/sqrt(dh) into Wq/bq; bv into bo
    (attn rows sum to 1); W passed pre-transposed [d, o] in bf16;
    coeff = softmax(rel_type_w)*sigmoid(rel_scale)*0.1.
  - LayerNorm natural (tokens on partitions, bn_stats/bn_aggr;
    rstd = exp(-0.5*ln(var+eps)) so every ACT call stays in the
    natural_log_exp table set -> single ACT table load for the whole kernel).
  - z tiles PE-transposed (bf16) so projections contract d on partitions:
    qT/kT produced [o, s]; v produced natural [s, o] into v_aug blocks of 65
    columns (64 v-cols + a ones column).
  - Relation bias: weighted^T computed once (DVE mul+reduce, PE transpose)
    and stored as exp(weighted^T) bf16, interleaved into the v-projection loop.
  - Per head: scoresT[k, q] on PE -> ACT exp straight from PSUM (bf16 out) ->
    DVE multiply by exp(weighted^T) per quad (exp(a+b) = exp(a)*exp(b)).
    The @v matmul contracts k with the ones column appended, so the softmax
    denominator Z lands in PSUM row 64 for free.  1/Z = exp(-ln Z) on ACT,
    replicated across partitions by a 1-deep (-1)-matmul.  attn rows and E
    are normalized on DVE; E goes straight to HBM as bf16.
  - Output projection consumes attn_concatT [d, q] with prefetched Wo^T.

Measured on trn2 (8 cores, axon): ~520-540 us per core, rel err ~5e-3
(bf16 quantization; fp32 LN stats and softmax accumulation throughout).
"""

import math
import os

import ml_dtypes
import numpy as np

# ---- problem constants (hardcoded by contract) -----------------------------
B, S, D = 2, 2048, 1024
H, DH = 16, 64
P = 128
NQB = 4  # q-blocks per batch -> 8 cores = B * NQB
QB = S // NQB  # 512
EPS = 1e-5
NCORES = 8

_PROGRAM_CACHE = {}
LAST_RESULTS = None  # BassKernelResults of most recent run (for test harness)


def _build_program():
    import concourse.bass as bass
    import concourse.mybir as mybir
    import concourse.tile as tile
    from concourse import bacc
    from concourse.masks import make_identity

    dt = mybir.dt
    Alu = mybir.AluOpType
    Act = mybir.ActivationFunctionType
    f32, bf16 = dt.float32, dt.bfloat16

    nc = bacc.Bacc(target_bir_lowering=False)

    # ---- I/O ---------------------------------------------------------------
    xq = nc.dram_tensor("xq", [QB, D], f32, kind="ExternalInput")
    xk = nc.dram_tensor("xk", [S, D], f32, kind="ExternalInput")
    xv = nc.dram_tensor("xv", [S, D], f32, kind="ExternalInput")
    tag = nc.dram_tensor("tag", [QB, S * 4], f32, kind="ExternalInput")
    wqT = nc.dram_tensor("wqT", [D, D], bf16, kind="ExternalInput")
    wkT = nc.dram_tensor("wkT", [D, D], bf16, kind="ExternalInput")
    wvT = nc.dram_tensor("wvT", [D, D], bf16, kind="ExternalInput")
    woT = nc.dram_tensor("woT", [D, D], bf16, kind="ExternalInput")
    bq = nc.dram_tensor("bq", [D], f32, kind="ExternalInput")
    bk = nc.dram_tensor("bk", [D], f32, kind="ExternalInput")
    bo = nc.dram_tensor("bo", [D], f32, kind="ExternalInput")
    coeff = nc.dram_tensor("coeff", [1, 4], f32, kind="ExternalInput")

    awT = nc.dram_tensor("awT", [H, S, QB], bf16, kind="ExternalOutput")
    attn_out = nc.dram_tensor("attn_out", [QB, D], f32, kind="ExternalOutput")

    VA = DH + 1  # v_aug block: 64 v-cols + ones column (fused softmax denom)

    with tile.TileContext(nc) as tc:
        with (
            tc.tile_pool(name="singles", bufs=1) as singles,
            tc.tile_pool(name="persist", bufs=1) as persist,
        ):
            # ---- constants -------------------------------------------------
            identity = singles.tile([P, P], bf16)
            make_identity(nc, identity)
            eps_t = singles.tile([P, 1], f32)
            nc.vector.memset(eps_t, EPS)
            minus1f = singles.tile([1, P], f32)
            nc.vector.memset(minus1f, -1.0)
            coeff_t = singles.tile([P, 4], f32)
            bq_t = singles.tile([P, 8], f32)
            bk_t = singles.tile([P, 8], f32)
            bo_t = singles.tile([P, D], f32)

            def load_constants():
                # issued after the first x-loads so the LayerNorm critical
                # path is not queued behind constant DMAs at startup
                nc.gpsimd.dma_start(
                    out=coeff_t,
                    in_=bass.AP(tensor=coeff, offset=0, ap=[[0, P], [1, 4]]),
                )
                nc.sync.dma_start(bq_t, bq[:].rearrange("(c p) -> p c", p=P))
                nc.sync.dma_start(bk_t, bk[:].rearrange("(c p) -> p c", p=P))
                nc.gpsimd.dma_start(
                    out=bo_t,
                    in_=bass.AP(tensor=bo, offset=0, ap=[[0, P], [1, D]]),
                )

            # ---- persistent SBUF tensors -----------------------------------
            expWT = persist.tile([P, 16 * QB], bf16)   # exp(weighted^T) (kchunk, q)
            qT_proj = persist.tile([P, 8 * QB], bf16)  # (ochunk, q)
            kT_proj = persist.tile([P, 8 * S], bf16)   # (ochunk, k)
            v_aug = persist.tile([P, 16 * H * VA], bf16)  # (ktile, head, 64+1)
            # only the per-head "ones" columns need initialising
            nc.vector.memset(
                v_aug[:].rearrange("p (t h a) -> p t h a", t=16, a=VA)[:, :, :, DH : DH + 1],
                1.0,
            )
            acT = persist.tile([P, 8 * QB], bf16)      # (dchunk, q)

            # ================= LN + q/k/v projections =======================
            def layernorm_tile(pool_x, pool_sm, x_dram, row0):
                x_t = pool_x.tile([P, D], f32, tag="x")
                nc.sync.dma_start(x_t, x_dram[row0 : row0 + P, :])
                stats = pool_sm.tile([P, 2, 6], f32, tag="stats")
                nc.vector.bn_stats(stats[:, 0, :], x_t[:, 0:512])
                nc.vector.bn_stats(stats[:, 1, :], x_t[:, 512:1024])
                mv = pool_sm.tile([P, 2], f32, tag="mv")
                nc.vector.bn_aggr(mv[:], stats[:])
                # rstd = exp(-0.5*ln(var+eps)): keeps every ACT call in the
                # natural_log_exp set -> no ACT table reloads anywhere
                lnv = pool_sm.tile([P, 1], f32, tag="lnv")
                nc.scalar.activation(lnv, mv[:, 1:2], Act.Ln, bias=eps_t[:, 0:1])
                rstd = pool_sm.tile([P, 1], f32, tag="rstd")
                nc.scalar.activation(rstd, lnv, Act.Exp, scale=-0.5)
                z_t = pool_x.tile([P, D], bf16, tag="z")
                nc.vector.tensor_scalar(
                    z_t[:], x_t[:], mv[:, 0:1], rstd[:, 0:1], Alu.subtract, Alu.mult
                )
                return z_t

            def transpose_into(zt, dest3, s_idx, psum_pool):
                for g in range(2):  # groups of 4 d-chunks per psum bank
                    pt = psum_pool.tile([P, 4, P], bf16, tag="pt")
                    for j in range(4):
                        nc.tensor.transpose(
                            pt[:, j, :],
                            zt[:, (4 * g + j) * P : (4 * g + j + 1) * P],
                            identity,
                        )
                    nc.scalar.copy(
                        dest3[:, 4 * g : 4 * g + 4, s_idx * P : (s_idx + 1) * P],
                        pt[:],
                    )

            with (
                tc.tile_pool(name="xp", bufs=2) as xp,
                tc.tile_pool(name="smp", bufs=4) as smp,
                tc.tile_pool(name="tagp", bufs=1) as tagp,
                tc.tile_pool(name="wnat", bufs=2) as wnatp,
                tc.tile_pool(name="wp", bufs=16) as wp,
                tc.tile_pool(name="zp", bufs=1) as zp,
                tc.tile_pool(name="zsl", bufs=2) as zsl,
                tc.tile_pool(name="psT", bufs=3, space="PSUM") as psT,
                tc.tile_pool(name="psP", bufs=3, space="PSUM") as psP,
            ):
                # ---- q ----
                zqT = zp.tile([P, 8 * QB], bf16)
                zqT3 = zqT[:].rearrange("p (c s) -> p c s", s=QB)
                for t in range(QB // P):  # 4
                    z_t = layernorm_tile(xp, smp, xq, t * P)
                    transpose_into(z_t, zqT3, t, psT)
                load_constants()
                # prefetch q projection weights (after the q x-loads, so the
                # first LayerNorms aren't queued behind weight DMAs); k/v
                # weights are staggered into the k-loop below for the same
                # reason (the x-loads for k slice 0 must not starve)
                wq_sb = [wp.tile([P, D], bf16, tag="w", name=f"wq{_i}") for _i in range(8)]
                wk_sb = [wp.tile([P, D], bf16, tag="w", name=f"wk{_i}") for _i in range(8)]
                for dc in range(8):
                    nc.sync.dma_start(wq_sb[dc], wqT[dc * P : (dc + 1) * P, :])
                for oc in range(8):
                    pp = psP.tile([P, QB], f32, tag="pp")
                    for dc in range(8):
                        nc.tensor.matmul(
                            pp,
                            wq_sb[dc][:, oc * P : (oc + 1) * P],
                            zqT[:, dc * QB : (dc + 1) * QB],
                            start=(dc == 0),
                            stop=(dc == 7),
                        )
                    nc.scalar.activation(
                        qT_proj[:, oc * QB : (oc + 1) * QB],
                        pp,
                        Act.Identity,
                        bias=bq_t[:, oc : oc + 1],
                    )

                # ---- tag relation bias -> expWT, interleaved with k below ----
                wT3 = expWT[:].rearrange("p (c q) -> p c q", q=QB)

                def tag_iter(qt):
                    tag_t = tagp.tile([P, S * 4], bf16, tag="tag")
                    nc.gpsimd.dma_start(out=tag_t[:], in_=tag[qt * P : (qt + 1) * P, :])
                    tag3 = tag_t[:].rearrange("p (s c) -> p s c", c=4)
                    w_nat = wnatp.tile([P, S], bf16, tag="wnat")
                    for half in range(2):
                        t3 = tag3[:, half * (S // 2) : (half + 1) * (S // 2), :]
                        cpat = coeff_t[:, None, :].to_broadcast((P, S // 2, 4))
                        nc.vector.tensor_tensor(t3, t3, cpat, Alu.mult)
                        with nc.allow_low_precision(reason="4-way add of small bf16 bias"):
                            nc.vector.tensor_reduce(
                                w_nat[:, half * (S // 2) : (half + 1) * (S // 2)],
                                t3,
                                axis=mybir.AxisListType.X,
                                op=Alu.add,
                            )
                    for g in range(4):  # groups of 4 kchunks per psum bank
                        pt = psT.tile([P, 4, P], bf16, tag="pt")
                        for j in range(4):
                            nc.tensor.transpose(
                                pt[:, j, :],
                                w_nat[:, (4 * g + j) * P : (4 * g + j + 1) * P],
                                identity,
                            )
                        nc.scalar.activation(
                            wT3[:, 4 * g : 4 * g + 4, qt * P : (qt + 1) * P],
                            pt[:],
                            Act.Exp,
                        )

                # ---- k ----
                wv_sb = [wp.tile([P, D], bf16, tag="w", name=f"wv{_i}") for _i in range(8)]
                for sl in range(S // 512):  # 4
                    zkT = zsl.tile([P, 8 * 512], bf16, tag="zk")
                    zkT3 = zkT[:].rearrange("p (c s) -> p c s", s=512)
                    for t in range(4):
                        z_t = layernorm_tile(xp, smp, xk, sl * 512 + t * P)
                        transpose_into(z_t, zkT3, t, psT)
                    if sl == 0:
                        for dc in range(8):
                            nc.sync.dma_start(wk_sb[dc], wkT[dc * P : (dc + 1) * P, :])
                    if sl == 1:
                        for dc in range(8):
                            nc.sync.dma_start(wv_sb[dc], wvT[dc * P : (dc + 1) * P, :])
                    for oc in range(8):
                        pp = psP.tile([P, 512], f32, tag="pp")
                        for dc in range(8):
                            nc.tensor.matmul(
                                pp,
                                wk_sb[dc][:, oc * P : (oc + 1) * P],
                                zkT[:, dc * 512 : (dc + 1) * 512],
                                start=(dc == 0),
                                stop=(dc == 7),
                            )
                        nc.scalar.activation(
                            kT_proj[:, oc * S + sl * 512 : oc * S + (sl + 1) * 512],
                            pp,
                            Act.Identity,
                            bias=bk_t[:, oc : oc + 1],
                        )

                # ---- v ---- (natural layout into v_aug; bv folded into bo on host)
                va3 = v_aug[:].rearrange("p (t h a) -> p (t h) a", t=16, a=VA)
                for kt in range(S // P):  # 16
                    if kt % 4 == 0:
                        tag_iter(kt // 4)
                    z_t = layernorm_tile(xp, smp, xv, kt * P)
                    zvT = zsl.tile([P, 8 * P], bf16, tag="zv")
                    zvT3 = zvT[:].rearrange("p (c s) -> p c s", s=P)
                    transpose_into(z_t, zvT3, 0, psT)
                    for oh in range(2):
                        pp = psP.tile([P, 512], f32, tag="pp")
                        for dc in range(8):
                            nc.tensor.matmul(
                                pp,
                                zvT[:, dc * P : (dc + 1) * P],
                                wv_sb[dc][:, oh * 512 : (oh + 1) * 512],
                                start=(dc == 0),
                                stop=(dc == 7),
                            )
                        nc.scalar.copy(
                            va3[:, kt * H + oh * 8 : kt * H + (oh + 1) * 8, 0:DH],
                            pp[:].rearrange("p (h a) -> p h a", a=DH),
                        )

            # ============ attention (incl. tag bias + output proj weights) ==
            NK = S // P  # 16 k-chunks
            wo_pool_cm = tc.tile_pool(name="wo_p", bufs=8)
            wo_p = wo_pool_cm.__enter__()
            with (
                tc.tile_pool(name="Ep", bufs=3) as Ep,
                tc.tile_pool(name="rzp", bufs=3) as rzp,
                tc.tile_pool(name="psS", bufs=2, space="PSUM") as psS,
                tc.tile_pool(name="psAV", bufs=3, space="PSUM") as psAV,
                tc.tile_pool(name="psA", bufs=1, space="PSUM") as psA,
            ):
                # prefetch output-projection weights during attention
                wo_sb = [wo_p.tile([P, D], bf16, tag="wo", name=f"wo{_i}") for _i in range(8)]
                for dc in range(8):
                    nc.sync.dma_start(wo_sb[dc], woT[dc * P : (dc + 1) * P, :])

                def head_front(h):
                    """scores -> exp -> *expWT -> @v, quad by quad.

                    Interleaving the @v accumulation into the scores loop means
                    the softmax denominator is ready right after the last exp,
                    so the ACT does not idle waiting for a monolithic @v block.
                    Returns (E tile, pav psum)."""
                    hp = 64 * (h % 2)
                    oc = h // 2
                    qT_h = qT_proj[hp : hp + 64, oc * QB : (oc + 1) * QB]
                    E = Ep.tile([P, NK * QB], bf16, tag="E", name=f"E{h}")
                    pav = psAV.tile([VA, QB], f32, tag="pav")
                    for g in range(8):  # pairs of k-chunks
                        ps = psS.tile([P, 2 * QB], f32, tag="ps")
                        for i in range(2):
                            c = 2 * g + i
                            nc.tensor.matmul(
                                ps[:, i * QB : (i + 1) * QB],
                                kT_proj[hp : hp + 64, oc * S + c * P : oc * S + (c + 1) * P],
                                qT_h,
                                start=True,
                                stop=True,
                            )
                        nc.scalar.activation(
                            E[:, 2 * g * QB : 2 * (g + 1) * QB], ps[:], Act.Exp
                        )
                        if g % 2 == 1:  # quad complete -> fold in exp(weighted)
                            q0 = (2 * g - 2) * QB
                            q1 = (2 * g + 2) * QB
                            nc.vector.tensor_tensor(
                                E[:, q0:q1], E[:, q0:q1], expWT[:, q0:q1], Alu.mult
                            )
                    return E, pav

                def head_tail(h, E, pav):
                    """@v + softmax denom + normalizations + output DMA."""
                    hp = 64 * (h % 2)
                    oc = h // 2
                    E3 = E[:].rearrange("p (c q) -> p c q", q=QB)
                    for c in range(NK):
                        nc.tensor.matmul(
                            pav,
                            v_aug[:, (c * H + h) * VA : (c * H + h + 1) * VA],
                            E[:, c * QB : (c + 1) * QB],
                            start=(c == 0),
                            stop=(c == NK - 1),
                        )
                    # 1/Z replicated across partitions: exp(-ln Z), broadcast
                    # via a tiny f32 ones-matmul
                    lnz1 = rzp.tile([1, QB], f32, tag="lnz1")
                    nc.scalar.activation(lnz1, pav[DH : DH + 1, :], Act.Ln)
                    rzPS = psA.tile([P, QB], f32, tag="rzPS")
                    nc.tensor.matmul(rzPS, minus1f, lnz1, start=True, stop=True)
                    rzb = rzp.tile([P, QB], bf16, tag="rzb")
                    nc.scalar.activation(rzb, rzPS, Act.Exp)
                    nc.vector.tensor_tensor(
                        acT[hp : hp + 64, oc * QB : (oc + 1) * QB],
                        pav[0:DH, :],
                        rzb[0:DH, :],
                        Alu.mult,
                    )
                    awT3 = awT[h].rearrange("(c p) q -> p c q", p=P)
                    half = NK // 2
                    for j in range(2):
                        sl = E3[:, j * half : (j + 1) * half, :]
                        nc.vector.tensor_tensor(
                            sl, sl, rzb[:, None, :].to_broadcast((P, half, QB)), Alu.mult
                        )
                        nc.sync.dma_start(awT3[:, j * half : (j + 1) * half, :], sl)

                for h in range(H):
                    E, pav = head_front(h)
                    head_tail(h, E, pav)

            # ================= output projection ============================
            with (
                tc.tile_pool(name="outp", bufs=3) as outp,
                tc.tile_pool(name="psO", bufs=2, space="PSUM") as psO,
            ):
                for qc in range(QB // P):  # 4
                    for oh in range(2):
                        po = psO.tile([P, 512], f32, tag="po")
                        for dc in range(8):
                            nc.tensor.matmul(
                                po,
                                acT[:, dc * QB + qc * P : dc * QB + (qc + 1) * P],
                                wo_sb[dc][:, oh * 512 : (oh + 1) * 512],
                                start=(dc == 0),
                                stop=(dc == 7),
                            )
                        o_sb = outp.tile([P, 512], f32, tag="osb")
                        nc.vector.tensor_tensor(
                            o_sb,
                            po,
                            bo_t[:, oh * 512 : (oh + 1) * 512],
                            Alu.add,
                        )
                        nc.sync.dma_start(
                            attn_out[qc * P : (qc + 1) * P, oh * 512 : (oh + 1) * 512],
                            o_sb,
                        )
            wo_pool_cm.__exit__(None, None, None)

    # Force Exp/Ln to resolve to natural_log_exp_and_others (the only set
    # holding both) so the ACT never reloads tables mid-kernel.  Set indices
    # must stay aligned with act_info.json, so edit contents, not order.
    import concourse.bacc as _bacc_mod
    from concourse.hw_specs import get_activation_tables as _gat

    def _gat_patched(arch):
        t = dict(_gat(arch))
        for name in ("exp_and_others", "exp_and_friends"):
            if name in t:
                t[name] = t[name] - {Act.Exp}
        if "natural_log" in t:
            t["natural_log"] = t["natural_log"] - {Act.Ln}
        return t

    orig = _bacc_mod.get_activation_tables
    _bacc_mod.get_activation_tables = _gat_patched
    try:
        nc.finalize()
    finally:
        _bacc_mod.get_activation_tables = orig
    return nc


def _get_program():
    if "nc" not in _PROGRAM_CACHE:
        _PROGRAM_CACHE["nc"] = _build_program()
    return _PROGRAM_CACHE["nc"]


def _install_axon_trace_support():
    """Register the NTFF-profile hook that concourse's axon trace path expects.

    The agent image lacks ``antenv.axon_hooks``; replicate trn_boot's ctypes
    hook against the local libaxon_pjrt.so.  Profiling-only; inert unless
    KERNEL_TRACE=1.
    """
    import contextlib
    import ctypes
    import sys
    import types

    if "antenv.axon_hooks" in sys.modules:
        return
    try:
        import antenv
    except ImportError:
        return
    so_path = "/opt/axon/libaxon_pjrt.so"
    if not os.path.exists(so_path):
        return
    lib = ctypes.CDLL(so_path)
    if not hasattr(lib, "axon_start_nrt_profile"):
        return
    lib.axon_start_nrt_profile.argtypes = [
        ctypes.POINTER(ctypes.c_int64),
        ctypes.c_size_t,
    ]
    lib.axon_start_nrt_profile.restype = ctypes.c_int64
    lib.axon_stop_nrt_profile.argtypes = [ctypes.c_char_p]
    lib.axon_stop_nrt_profile.restype = ctypes.c_int64

    @contextlib.contextmanager
    def _hook(output_dir, device_ids):
        import jax

        jax.devices()
        if device_ids:
            ids = (ctypes.c_int64 * len(device_ids))(*device_ids)
            rc = lib.axon_start_nrt_profile(ids, len(device_ids))
        else:
            rc = lib.axon_start_nrt_profile(None, 0)
        if rc != 0:
            raise RuntimeError(f"axon_start_nrt_profile rc={rc}")
        try:
            yield
        finally:
            n = lib.axon_stop_nrt_profile(str(output_dir).encode())
            print(f"ntff profile: {n} file(s) written to {output_dir}")

    hooks = types.ModuleType("antenv.axon_hooks")
    _store = {"h": _hook}
    hooks.set_axon_ntff_profile_hook = lambda h: _store.__setitem__("h", h)
    hooks.get_axon_ntff_profile_hook = lambda: _store["h"]
    sys.modules["antenv.axon_hooks"] = hooks
    antenv.axon_hooks = hooks

    # avoid S3 artifact uploads from the profile path in this container
    import concourse.bass_utils as bu

    bu.upload_artifacts = lambda tmpdir: tmpdir


def kernel(**inputs):
    global LAST_RESULTS
    from concourse.bass_utils import run_bass_kernel_spmd

    q = np.asarray(inputs["query"], np.float32)
    k = np.asarray(inputs["key"], np.float32)
    v = np.asarray(inputs["value"], np.float32)
    tag = np.asarray(inputs["tag_relations"], np.float32)
    gamma = np.asarray(inputs["ln_gamma"], np.float32)
    beta = np.asarray(inputs["ln_beta"], np.float32)
    Wq = np.asarray(inputs["Wq"], np.float32)
    Wk = np.asarray(inputs["Wk"], np.float32)
    Wv = np.asarray(inputs["Wv"], np.float32)
    Wo = np.asarray(inputs["Wo"], np.float32)
    bq = np.asarray(inputs["bq"], np.float32)
    bk = np.asarray(inputs["bk"], np.float32)
    bv = np.asarray(inputs["bv"], np.float32)
    bo = np.asarray(inputs["bo"], np.float32)
    rel_type_w = np.asarray(inputs["rel_type_w"], np.float32)
    rel_scale = np.asarray(inputs["rel_scale"], np.float32)
    # NOTE: mask is all-ones per the problem spec (fill: ones) -> no-op.

    scale = 1.0 / math.sqrt(DH)
    # fold LN gamma/beta and the 1/sqrt(dh) scale into weights/biases (host-side
    # static param prep; gamma/beta are per-feature so they fold exactly).
    f64 = np.float64
    bf16 = ml_dtypes.bfloat16
    wqT = np.ascontiguousarray((gamma[:, None] * Wq.T.astype(f64)) * scale).astype(bf16)
    bq_e = np.asarray((bq + Wq.astype(f64) @ beta) * scale, np.float32)
    wkT = np.ascontiguousarray(gamma[:, None] * Wk.T.astype(f64)).astype(bf16)
    bk_e = np.asarray(bk + Wk.astype(f64) @ beta, np.float32)
    wvT = np.ascontiguousarray(gamma[:, None] * Wv.T.astype(f64)).astype(bf16)
    bv_e = np.asarray(bv + Wv.astype(f64) @ beta, f64)
    woT = np.ascontiguousarray(Wo.T).astype(bf16)
    # attn weights sum to 1, so the (v + bv) bias passes through attention
    # unchanged and folds into the output bias: bo_eff = bo + Wo @ bv
    bo = np.asarray(bo + Wo.astype(f64) @ bv_e, np.float32)

    rw = np.exp(rel_type_w - rel_type_w.max())
    rw /= rw.sum()
    coeff = (rw * (1.0 / (1.0 + np.exp(-rel_scale))) * 0.1).astype(np.float32)
    coeff = np.ascontiguousarray(coeff.reshape(1, 4))

    shared = dict(
        wqT=wqT, wkT=wkT, wvT=wvT, woT=woT,
        bq=bq_e, bk=bk_e, bo=bo, coeff=coeff,
    )
    in_maps = []
    for core in range(NCORES):
        b, j = divmod(core, NQB)
        in_maps.append(
            dict(
                shared,
                xq=np.ascontiguousarray(q[b, j * QB : (j + 1) * QB]),
                xk=np.ascontiguousarray(k[b]),
                xv=np.ascontiguousarray(v[b]),
                tag=np.ascontiguousarray(
                    tag[b, j * QB : (j + 1) * QB].reshape(QB, S * 4)
                ),
            )
        )

    nc = _get_program()
    trace = bool(int(os.environ.get("KERNEL_TRACE", "0")))
    if trace:
        _install_axon_trace_support()
    res = run_bass_kernel_spmd(
        nc, in_maps, core_ids=list(range(NCORES)), trace=trace
    )
    LAST_RESULTS = res

    attn_output = np.empty((B, S, D), np.float32)
    attn_weights = np.empty((B, H, S, S), np.float32)
    for core in range(NCORES):
        b, j = divmod(core, NQB)
        r = res.results[core]
        attn_output[b, j * QB : (j + 1) * QB, :] = r["attn_out"]
        # awT[h, k, q] (bf16) -> [h, q, k] fp32
        attn_weights[b, :, j * QB : (j + 1) * QB, :] = (
            r["awT"].astype(np.float32).swapaxes(1, 2)
        )
    return attn_output, attn_weights
